# revision 45
# baseline (speedup 1.0000x reference)
"""BertSelfAttention on 8 Trainium2 NeuronCores.

Sharding: tensor-parallel over heads. Core c computes heads 2c and 2c+1,
i.e. output columns [c*128, (c+1)*128) of the [B*S, D] context output.

Per-core dataflow (all matmul operands bf16, fp32 PSUM accumulation):
  per batch b (phases pipelined across batches by the Tile scheduler):
    phase 1: Q^T, K^T = W^T @ X^T (X^T pretransposed on host), V = X @ Wv
             stored with a ones column per head so the context matmul also
             produces softmax denominators.
    phase 2: S^T = K @ Q^T for both heads packed into disjoint PE row
             groups (concurrent), P^T = exp(S^T/8 + mask) on ScalarE over
             [128,1024] tiles, ctx_aug^T = [V|1]^T @ P^T accumulated over
             key chunks, PE-transpose, per-row normalize, DMA out.
"""

import os
import sys

import numpy as np

try:
    import concourse  # noqa: F401  (normally provided by the environment)
except ImportError:  # pragma: no cover - fallback for bare containers
    for _p in ("/opt/trn_rl_repo", "/root/.axon_site/_ro/trn_rl_repo"):
        if os.path.isdir(_p) and _p not in sys.path:
            sys.path.append(_p)

import ml_dtypes

B, S, D, H = 4, 2048, 1024, 16
HD = D // H          # 64
NCORES = 8
HPC = H // NCORES    # heads per core = 2
CW = HPC * HD        # output columns per core = 128
BS = B * S           # 8192
SBPB = S // 512      # seq blocks per batch (phase 1) = 4
NKC = S // 128       # key chunks per batch = 16
VW = CW + HPC        # V chunk width with ones columns = 130

_CACHE = {}


def _build():
    import concourse.mybir as mybir
    import concourse.tile as tile
    from concourse import bacc

    f32 = mybir.dt.float32
    bf16 = mybir.dt.bfloat16
    Exp = mybir.ActivationFunctionType.Exp
    inv_sqrt_hd = 1.0 / float(np.sqrt(HD))

    nc = bacc.Bacc(None)

    xt = nc.declare_dram_parameter("xt", [D, BS], bf16, isOutput=False)
    wq = nc.declare_dram_parameter("wq", [D, CW], bf16, isOutput=False)
    wk = nc.declare_dram_parameter("wk", [D, CW], bf16, isOutput=False)
    wv = nc.declare_dram_parameter("wv", [D, CW], bf16, isOutput=False)
    bq = nc.declare_dram_parameter("bq", [CW, 1], f32, isOutput=False)
    bk = nc.declare_dram_parameter("bk", [CW, 1], f32, isOutput=False)
    bvb = nc.declare_dram_parameter("bvb", [128, CW], f32, isOutput=False)
    maskp = nc.declare_dram_parameter("maskp", [128, B * NKC], f32, isOutput=False)
    ident = nc.declare_dram_parameter("ident", [128, 128], f32, isOutput=False)
    out = nc.declare_dram_parameter("out", [BS, CW], f32, isOutput=True)

    with tile.TileContext(nc) as tc:
        with (
            tc.tile_pool(name="const", bufs=1) as cp,
            tc.tile_pool(name="xt", bufs=4) as xp,
            tc.tile_pool(name="p1", bufs=2, space="PSUM") as p1,
            tc.tile_pool(name="sps", bufs=2, space="PSUM") as sp,
            tc.tile_pool(name="ctx", bufs=2, space="PSUM") as cxp,
            tc.tile_pool(name="pt", bufs=6) as ptp,
            tc.tile_pool(name="fin", bufs=4) as fp,
        ):
            wq_sb = cp.tile([128, D], bf16, tag="wq")
            wk_sb = cp.tile([128, D], bf16, tag="wk")
            wv_sb = cp.tile([128, D], bf16, tag="wv")
            # First seq block's X^T loads interleaved with the weight chunks
            # so the first projection matmuls start early.
            xt_t0 = xp.tile([128, 8 * 512], bf16, tag="xt", name="xt_0_0")

            def load_xt(dst, gs):
                for dc in range(8):
                    nc.sync.dma_start(
                        dst[:, dc * 512 : (dc + 1) * 512],
                        xt[dc * 128 : (dc + 1) * 128, gs],
                    )

            for dc in range(8):
                cs = slice(dc * 128, (dc + 1) * 128)
                nc.sync.dma_start(wq_sb[:, cs], wq[cs, :])
                nc.sync.dma_start(
                    xt_t0[:, dc * 512 : (dc + 1) * 512],
                    xt[dc * 128 : (dc + 1) * 128, 0:512],
                )
                nc.sync.dma_start(wk_sb[:, cs], wk[cs, :])
                nc.sync.dma_start(wv_sb[:, cs], wv[cs, :])
            bq_sb = cp.tile([CW, 1], f32, tag="bq")
            bk_sb = cp.tile([CW, 1], f32, tag="bk")
            bvb_sb = cp.tile([128, CW], f32, tag="bvb")
            mask_sb = cp.tile([128, B * NKC], f32, tag="mask")
            id_sb = cp.tile([128, 128], f32, tag="ident")
            nc.sync.dma_start(bq_sb[:], bq[:])
            nc.sync.dma_start(bk_sb[:], bk[:])
            nc.sync.dma_start(bvb_sb[:], bvb[:])
            nc.sync.dma_start(mask_sb[:], maskp[:])
            nc.sync.dma_start(id_sb[:], ident[:])

            # Per-seq-block tiles so phase 2 can chase phase 1 block-by-block
            # (dependencies stay fine-grained regardless of Tile's range
            # tracking granularity).
            qt = {}
            kt = {}
            vt = {}
            for b in range(B):
                for j in range(SBPB):
                    qt[b, j] = cp.tile([128, 512], bf16, tag=f"qt{b}_{j}", name=f"qt{b}_{j}")
                    kt[b, j] = cp.tile([128, 512], bf16, tag=f"kt{b}_{j}", name=f"kt{b}_{j}")
                    vt[b, j] = cp.tile([128, 4 * VW], bf16, tag=f"vt{b}_{j}", name=f"vt{b}_{j}")
                    nc.gpsimd.memset(vt[b, j][:], 1.0)

            def phase1(b):
                # ---------------- phase 1(b): projections ----------------
                for sb in range(SBPB):
                    gs = slice((b * SBPB + sb) * 512, (b * SBPB + sb + 1) * 512)
                    if b == 0 and sb == 0:
                        xt_t = xt_t0
                    else:
                        xt_t = xp.tile(
                            [128, 8 * 512], bf16, tag="xt", name=f"xt_{b}_{sb}"
                        )
                        load_xt(xt_t, gs)
                    psq = p1.tile([128, 512], f32, tag="p1", name=f"psq_{b}_{sb}")
                    for dc in range(8):
                        nc.tensor.matmul(
                            psq[:],
                            lhsT=wq_sb[:, dc * 128 : (dc + 1) * 128],
                            rhs=xt_t[:, dc * 512 : (dc + 1) * 512],
                            start=(dc == 0),
                            stop=(dc == 7),
                        )
                    nc.vector.tensor_scalar_add(qt[b, sb][:], psq[:], bq_sb[:])
                    psk = p1.tile([128, 512], f32, tag="p1", name=f"psk_{b}_{sb}")
                    for dc in range(8):
                        nc.tensor.matmul(
                            psk[:],
                            lhsT=wk_sb[:, dc * 128 : (dc + 1) * 128],
                            rhs=xt_t[:, dc * 512 : (dc + 1) * 512],
                            start=(dc == 0),
                            stop=(dc == 7),
                        )
                    nc.vector.tensor_scalar_add(kt[b, sb][:], psk[:], bk_sb[:])
                    for t in range(4):
                        psv = p1.tile([128, 128], f32, tag="p1", name=f"psv_{b}_{sb}_{t}")
                        for dc in range(8):
                            nc.tensor.matmul(
                                psv[:],
                                lhsT=xt_t[:, dc * 512 + t * 128 : dc * 512 + (t + 1) * 128],
                                rhs=wv_sb[:, dc * 128 : (dc + 1) * 128],
                                start=(dc == 0),
                                stop=(dc == 7),
                            )
                        nc.vector.tensor_copy(
                            vt[b, sb][:, t * VW : t * VW + HD], psv[:, 0:HD]
                        )
                        nc.vector.tensor_copy(
                            vt[b, sb][:, t * VW + HD + 1 : t * VW + 2 * HD + 1],
                            psv[:, HD : 2 * HD],
                        )

            def phase2(b):
                # ---------------- phase 2(b): attention ----------------
                for qb in range(4):
                    q0 = qb * 512
                    ctxs = [
                        cxp.tile([HD + 1, 512], f32, tag="ctx", name=f"ctx_{b}_{qb}_{h}")
                        for h in range(HPC)
                    ]
                    pts = {}
                    for kc in range(NKC + 1):
                        if kc < NKC:
                            kb, ko = kc // 4, (kc % 4) * 128
                            sps_t = sp.tile([128, 1024], f32, tag="s", name=f"s_{b}_{qb}_{kc}")
                            for h in range(HPC):
                                hs = slice(h * HD, (h + 1) * HD)
                                nc.tensor.matmul(
                                    sps_t[:, h * 512 : (h + 1) * 512],
                                    lhsT=kt[b, kb][hs, ko : ko + 128],
                                    rhs=qt[b, qb][hs, :],
                                    start=True,
                                    stop=True,
                                )
                            pt_t = ptp.tile([128, 1024], bf16, tag="pt", name=f"pt_{b}_{qb}_{kc}")
                            nc.scalar.activation(
                                pt_t[:],
                                sps_t[:],
                                Exp,
                                bias=mask_sb[:, b * NKC + kc : b * NKC + kc + 1],
                                scale=inv_sqrt_hd,
                            )
                            pts[kc] = pt_t
                        if kc >= 1:
                            pkc = kc - 1
                            pkb, pko = pkc // 4, (pkc % 4) * VW
                            for h in range(HPC):
                                nc.tensor.matmul(
                                    ctxs[h][:],
                                    lhsT=vt[b, pkb][:, pko + h * (HD + 1) : pko + (h + 1) * (HD + 1)],
                                    rhs=pts[pkc][:, h * 512 : (h + 1) * 512],
                                    start=(pkc == 0),
                                    stop=(pkc == NKC - 1),
                                )
                            del pts[pkc]
                    for h in range(HPC):
                        hs = slice(h * HD, (h + 1) * HD)
                        csb = fp.tile([HD + 1, 512], f32, tag="csb", name=f"csb_{b}_{qb}_{h}")
                        nc.vector.tensor_copy(csb[:], ctxs[h][:])
                        for t in range(4):
                            tr = cxp.tile([128, HD + 1], f32, tag="ctx", name=f"tr_{b}_{qb}_{h}_{t}")
                            nc.tensor.transpose(
                                tr[:],
                                csb[:, t * 128 : (t + 1) * 128],
                                id_sb[0 : HD + 1, 0 : HD + 1],
                            )
                            rcp = fp.tile([128, 1], f32, tag="rcp", bufs=8, name=f"rcp_{b}_{qb}_{h}_{t}")
                            nc.vector.reciprocal(rcp[:], tr[:, HD : HD + 1])
                            osb = fp.tile([128, HD], f32, tag="osb", bufs=8, name=f"osb_{b}_{qb}_{h}_{t}")
                            nc.vector.tensor_scalar_mul(osb[:], tr[:, 0:HD], rcp[:])
                            nc.vector.tensor_add(osb[:], osb[:], bvb_sb[:, hs])
                            r0 = b * S + q0 + t * 128
                            nc.sync.dma_start(out[r0 : r0 + 128, hs], osb[:])

            # Plain per-batch emission wins: attention (which feeds the
            # ACT-bound exp stream) keeps scheduler priority, and the next
            # batch's projections fill PE gaps on their own.
            for b in range(B):
                phase1(b)
                phase2(b)

    nc.finalize()
    return nc


def _get_nc():
    if "nc" not in _CACHE:
        _CACHE["nc"] = _build()
    return _CACHE["nc"]


def make_in_maps(hidden_states, attention_mask, Wq, bq, Wk, bk, Wv, bv):
    X = np.asarray(hidden_states, np.float32).reshape(BS, D)
    XT = np.ascontiguousarray(X.T).astype(ml_dtypes.bfloat16)
    mask = np.asarray(attention_mask, np.float32).reshape(B, S)
    maskp = np.ascontiguousarray(
        mask.reshape(B, NKC, 128).transpose(2, 0, 1).reshape(128, B * NKC)
    )
    identity = np.eye(128, dtype=np.float32)
    Wq = np.asarray(Wq, np.float32)
    Wk = np.asarray(Wk, np.float32)
    Wv = np.asarray(Wv, np.float32)
    bqf = np.asarray(bq, np.float32)
    bkf = np.asarray(bk, np.float32)
    bvf = np.asarray(bv, np.float32)

    in_maps = []
    for c in range(NCORES):
        sl = slice(c * CW, (c + 1) * CW)
        in_maps.append(
            {
                "xt": XT,
                "wq": Wq[:, sl].astype(ml_dtypes.bfloat16),
                "wk": Wk[:, sl].astype(ml_dtypes.bfloat16),
                "wv": Wv[:, sl].astype(ml_dtypes.bfloat16),
                "bq": np.ascontiguousarray(bqf[sl].reshape(CW, 1)),
                "bk": np.ascontiguousarray(bkf[sl].reshape(CW, 1)),
                "bvb": np.ascontiguousarray(
                    np.broadcast_to(bvf[sl], (128, CW)).astype(np.float32)
                ),
                "maskp": maskp,
                "ident": identity,
            }
        )
    return in_maps


def kernel(hidden_states, attention_mask, Wq, bq, Wk, bk, Wv, bv):
    from concourse.bass_utils import run_bass_kernel_spmd

    nc = _get_nc()
    in_maps = make_in_maps(
        hidden_states, attention_mask, Wq, bq, Wk, bk, Wv, bv
    )
    res = run_bass_kernel_spmd(nc, in_maps, list(range(NCORES)))
    outs = [res.results[c]["out"] for c in range(NCORES)]
    return np.concatenate(outs, axis=1).reshape(B, S, D)
